# revision 1
# baseline (speedup 1.0000x reference)
# Trainium2 Bass kernel for nn_BDHBlock (dense transformer block).
#
# Strategy (8 NeuronCores, one shared SPMD program):
#   - Token-parallel for all token-local stages: core c owns flat tokens
#     [512c, 512c+512) of x.reshape(4096, 1024). LayerNorms, the masked
#     sparse linear, QKV / output projections and the FFN are computed
#     locally with replicated (host pre-transposed) weights.
#   - Attention is head-parallel: an AllToAll reshards q/k/v from
#     token-sharded to head-sharded (2 heads x full 4096-token sequence per
#     core), each core runs exact-causal relu attention for its 2 heads,
#     and a second AllToAll reshards the context back to token-sharded.
#     This keeps the program identical on every core (static loops).
#   - Matmul dtypes: float32r (full-rate fp32) for all weight-stationary
#     linears; fp16 for attention and ff2 (w2 cast on-chip after a f32 load).
import numpy as np

import concourse.bass as bass
import concourse.mybir as mybir
import concourse.tile as tile
from concourse import bacc
from concourse.masks import make_identity

B, S, H, NH = 2, 2048, 1024, 16
D = H // NH            # 64
FF = 4 * H             # 4096
NC = 8                 # cores
T = B * S // NC        # 512 tokens per core
TT = T // 128          # 4 token tiles
KT = H // 128          # 8 feature tiles
HPC = 2                # heads per core
F32, F32R, F16 = mybir.dt.float32, mybir.dt.float32r, mybir.dt.float16
ADD, SUB, MUL, MAX = (mybir.AluOpType.add, mybir.AluOpType.subtract,
                      mybir.AluOpType.mult, mybir.AluOpType.max)
AF = mybir.ActivationFunctionType
RG = [list(range(NC))]
EPS = 1e-5

_CACHE = {}


def _r(ap):
    return ap.bitcast(F32R)


def _build():
    nc = bacc.Bacc("TRN2", target_bir_lowering=False, debug=False,
                   num_devices=NC)

    # ---------------- I/O ----------------
    def inp(name, shape, dtype=F32):
        return nc.dram_tensor(name, list(shape), dtype, kind="ExternalInput")

    x_io = inp("x_c", (T, H))
    sfwT_io = inp("sfwT", (H, H))
    maskT_io = inp("maskT", (H, H))
    wT_io = {k: inp(k, (H, H)) for k in ("wqT", "wkT", "wvT", "woT")}
    w1T_io = inp("w1T", (H, FF))
    w2T_io = inp("w2T", (FF, H))
    b_io = {k: inp(k, (H,)) for k in ("sf_b", "bq", "bk", "bv", "bo", "ff2_b")}
    ff1b_io = inp("ff1_b", (FF,))
    gb_io = {k: inp(k, (H,)) for k in ("g1", "b1", "g2", "b2", "g3", "b3")}
    tri_io = inp("tri", (2, 128, 256))           # fp32 diag masks
    bqk_col_io = inp("bqk_col", (128, 2 * KT))   # [p, 2*kt]: bq/bk per-partition cols
    ff1b_col_io = inp("ff1b_col", (128, FF // 128))
    out_io = nc.dram_tensor("out_c", [T, H], F32, kind="ExternalOutput")

    # internal DRAM for collectives (HBM bounce; out must be Shared)
    SLOT = 128 * T                               # elements per (dest, tensor) slot
    kv_in = nc.dram_tensor("kv_in", [NC, 2, SLOT], F16)
    kv_out = nc.dram_tensor("kv_out", [NC, 2, SLOT], F16)
    q_in = nc.dram_tensor("q_in", [NC, SLOT], F16)
    q_out = nc.dram_tensor("q_out", [NC, SLOT], F16)
    cc_in = nc.dram_tensor("cc_in", [NC, SLOT], F16)
    cc_out = nc.dram_tensor("cc_out", [NC, SLOT], F16)

    from contextlib import ExitStack
    with tile.TileContext(nc) as tc, ExitStack() as es:
        # ---------------- pools ----------------
        const = es.enter_context(tc.tile_pool(name="const", bufs=1))
        persist = es.enter_context(tc.tile_pool(name="persist", bufs=1))
        wpool = es.enter_context(tc.tile_pool(name="wpool", bufs=6))  # f32 [128,512]
        wpool16 = es.enter_context(tc.tile_pool(name="wpool16", bufs=4))  # f16 weight tiles
        sc_pool = es.enter_context(tc.tile_pool(name="scratch", bufs=3))
        small = es.enter_context(tc.tile_pool(name="small", bufs=8))
        pacc = es.enter_context(tc.tile_pool(name="pacc", bufs=1, space="PSUM"))  # 4 acc tags = 4 banks
        pmix = es.enter_context(tc.tile_pool(name="pmix", bufs=4, space="PSUM"))  # shared rotating tag = 3 banks

        ident = const.tile([128, 128], F32)
        make_identity(nc, ident)
        tri = const.tile([128, 2, 256], F32)
        nc.sync.dma_start(out=tri[:], in_=tri_io.ap().rearrange("a p q -> p a q"))
        bqk_col = const.tile([128, 2 * KT], F32)
        nc.sync.dma_start(out=bqk_col[:], in_=bqk_col_io.ap())
        ff1b_col = const.tile([128, FF // 128], F32)
        nc.sync.dma_start(out=ff1b_col[:], in_=ff1b_col_io.ap())
        eps_col = const.tile([128, 1], F32)
        nc.vector.memset(eps_col[:], EPS)
        ones64 = const.tile([1, 64], F32)
        nc.vector.memset(ones64[:], 1.0)

        _round = [0]
        def acc_tiles():
            r = _round[0]; _round[0] += 1
            if r % 2 == 0:
                return [pacc.tile([128, 512], F32, tag=f"acc{t}", name=f"acc{t}")
                        for t in range(4)]
            return [pmix.tile([128, 512], F32, tag="pmix", name=f"accp{t}")
                    for t in range(4)]

        # residual stream, token-major [128, tt, H]
        x_sb = persist.tile([128, TT, H], F32)
        nc.sync.dma_start(out=x_sb[:], in_=x_io.ap().rearrange("(tt p) h -> p tt h", p=128))

        # slot-sharing tags: bigA = lnT (16K) then h (32K); bigB = qT (8K) then
        # ctxT (16K); bigC = kT then ctxo (8K); bigD = v then ln3T (8K)
        lnT_sb = persist.tile([128, KT, T], F32R, tag="bigA")
        qT_sb = persist.tile([128, KT, T], F16, tag="bigB")
        kT_sb = persist.tile([128, KT, T], F16, tag="bigC")
        v_sb = persist.tile([128, TT, H], F16, tag="bigD")
        g_bc = persist.tile([128, H], F32)              # gamma broadcast scratch
        beta_bc = persist.tile([128, H], F32)           # beta broadcast scratch
        bias_bc = persist.tile([128, H], F32)           # free-dim bias broadcast scratch

        row_pool = es.enter_context(tc.tile_pool(name="rows", bufs=1))

        def bcast_row(dst, src_dram, n):
            """Broadcast a [n] DRAM row across 128 partitions via zero-stride DMA."""
            src = src_dram.ap().unsqueeze(0).partition_broadcast(128).squeeze(1)
            nc.gpsimd.dma_start(out=dst[:, :n], in_=src)

        # ---------------- layernorm (token-major) + transpose ----------------
        def layer_norm_t(g_name, b_name, dst):
            """LN over x_sb tokens; writes transposed output into dst [128, kt, T]."""
            bcast_row(g_bc, gb_io[g_name], H)
            bcast_row(beta_bc, gb_io[b_name], H)
            for tt in range(TT):
                xt = x_sb[:, tt, :]
                sums = small.tile([128, 1], F32, tag="s0")
                sumsq = small.tile([128, 1], F32, tag="s1")
                lt = sc_pool.tile([128, H], F32, tag="lnt")
                nc.vector.reduce_sum(sums[:], xt, axis=mybir.AxisListType.X)
                nc.scalar.activation(lt[:], xt, AF.Square, accum_out=sumsq[:])
                mu = small.tile([128, 1], F32, tag="s2")
                var = small.tile([128, 1], F32, tag="s3")
                rstd = small.tile([128, 1], F32, tag="s4")
                nc.vector.tensor_scalar_mul(mu[:], sums[:], 1.0 / H)
                nc.vector.tensor_scalar_mul(var[:], sumsq[:], 1.0 / H)
                nc.vector.tensor_tensor(rstd[:], mu[:], mu[:], MUL)
                nc.vector.tensor_tensor(var[:], var[:], rstd[:], SUB)
                nc.scalar.activation(rstd[:], var[:], AF.Sqrt, bias=eps_col[:])
                nc.vector.reciprocal(rstd[:], rstd[:])
                nc.vector.tensor_scalar(lt[:], xt, mu[:], rstd[:], op0=SUB, op1=MUL)
                nc.any.tensor_mul(lt[:], lt[:], g_bc[:])
                nc.any.tensor_add(lt[:], lt[:], beta_bc[:])
                for kt in range(KT):
                    pt = pmix.tile([128, 512], F32, tag="pmix", name="pt")
                    nc.tensor.transpose(pt[:, :128], lt[:, bass.ts(kt, 128)], ident[:])
                    nc.any.tensor_copy(dst[:, kt, bass.ts(tt, 128)], pt[:, :128])

        # =====================================================================
        # Stage 1: x += LN1(x) @ (sf_w * mask).T + sf_b
        # =====================================================================
        layer_norm_t("g1", "b1", lnT_sb)
        bcast_row(bias_bc, b_io["sf_b"], H)
        for nch in range(2):
            ps = acc_tiles()
            for kt in range(KT):
                wt = wpool.tile([128, 512], F32R, tag="wa")
                mt = wpool.tile([128, 512], F32, tag="wb")
                nc.sync.dma_start(out=wt[:], in_=_r(sfwT_io.ap()[bass.ts(kt, 128), bass.ts(nch, 512)]))
                nc.sync.dma_start(out=mt[:], in_=maskT_io.ap()[bass.ts(kt, 128), bass.ts(nch, 512)])
                nc.any.tensor_mul(wt[:], wt[:], mt[:])
                for tt in range(TT):
                    nc.tensor.matmul(ps[tt][:], lnT_sb[:, kt, bass.ts(tt, 128)],
                                     wt[:], start=(kt == 0), stop=(kt == KT - 1))
            for tt in range(TT):
                xsl = x_sb[:, tt, bass.ts(nch, 512)]
                tmp = sc_pool.tile([128, 512], F32, tag="ev")
                nc.any.tensor_add(tmp[:], ps[tt][:], bias_bc[:, bass.ts(nch, 512)])
                nc.any.tensor_add(xsl, xsl, tmp[:])

        # =====================================================================
        # Stage 2: LN2 + QKV
        # =====================================================================
        layer_norm_t("g2", "b2", lnT_sb)
        # q/k: feature-major out [n 128, t 512]; scale q/k by 1/sqrt(sqrt(D)) each
        # so scores come out pre-scaled by 1/sqrt(D).
        qsc = 1.0 / float(np.sqrt(np.sqrt(D)))
        for wio, dst, bcol in [("wkT", kT_sb, 1)]:
            for nh in range(2):
                ps = acc_tiles()
                for kt in range(KT):
                    wt = wpool.tile([128, 512], F32R, tag="wa")
                    nc.sync.dma_start(out=wt[:], in_=_r(wT_io[wio].ap()[bass.ts(kt, 128), bass.ts(nh, 512)]))
                    for n4 in range(4):
                        nc.tensor.matmul(ps[n4][:], wt[:, bass.ts(n4, 128)], lnT_sb[:, kt, :],
                                         start=(kt == 0), stop=(kt == KT - 1))
                for n4 in range(4):
                    nt = nh * 4 + n4
                    nc.any.tensor_scalar(dst[:, nt, :], ps[n4][:],
                                         bqk_col[:, bcol * KT + nt: bcol * KT + nt + 1],
                                         qsc, op0=ADD, op1=MUL)
        # v: token-major out [t 128, n 512]
        bcast_row(bias_bc, b_io["bv"], H)
        for nch in range(2):
            ps = acc_tiles()
            for kt in range(KT):
                wt = wpool.tile([128, 512], F32R, tag="wa")
                nc.sync.dma_start(out=wt[:], in_=_r(wT_io["wvT"].ap()[bass.ts(kt, 128), bass.ts(nch, 512)]))
                for tt in range(TT):
                    nc.tensor.matmul(ps[tt][:], lnT_sb[:, kt, bass.ts(tt, 128)],
                                     wt[:], start=(kt == 0), stop=(kt == KT - 1))
            for tt in range(TT):
                nc.any.tensor_add(v_sb[:, tt, bass.ts(nch, 512)], ps[tt][:],
                                  bias_bc[:, bass.ts(nch, 512)])

        # =====================================================================
        # A2A #1a: k/v exchange (overlaps with q production below)
        # =====================================================================
        for j in range(NC):
            nc.sync.dma_start(out=kv_in.ap()[j, 0].rearrange("(p t) -> p t", p=128),
                              in_=kT_sb[:, j, :])
            nc.sync.dma_start(out=kv_in.ap()[j, 1].rearrange("(p tt f) -> p tt f", p=128, tt=TT),
                              in_=v_sb[:, :, bass.ts(j, 128)])
        nc.gpsimd.collective_compute(
            "AllToAll", mybir.AluOpType.bypass, replica_groups=RG,
            ins=[kv_in.ap().opt()], outs=[kv_out.ap().opt()])
        # q production (overlaps the kv A2A)
        for wio, dst, bcol in [("wqT", qT_sb, 0)]:
            for nh in range(2):
                ps = acc_tiles()
                for kt in range(KT):
                    wt = wpool.tile([128, 512], F32R, tag="wa")
                    nc.sync.dma_start(out=wt[:], in_=_r(wT_io[wio].ap()[bass.ts(kt, 128), bass.ts(nh, 512)]))
                    for n4 in range(4):
                        nc.tensor.matmul(ps[n4][:], wt[:, bass.ts(n4, 128)], lnT_sb[:, kt, :],
                                         start=(kt == 0), stop=(kt == KT - 1))
                for n4 in range(4):
                    nt = nh * 4 + n4
                    nc.any.tensor_scalar(dst[:, nt, :], ps[n4][:],
                                         bqk_col[:, bcol * KT + nt: bcol * KT + nt + 1],
                                         qsc, op0=ADD, op1=MUL)
        for j in range(NC):
            nc.sync.dma_start(out=q_in.ap()[j].rearrange("(p t) -> p t", p=128),
                              in_=qT_sb[:, j, :])
        nc.gpsimd.collective_compute(
            "AllToAll", mybir.AluOpType.bypass, replica_groups=RG,
            ins=[q_in.ap().opt()], outs=[q_out.ap().opt()])

        # =====================================================================
        # Attention: 2 heads, full sequence, exact causal
        # =====================================================================
        ctxT_sb = persist.tile([128, B, S], F16, tag="bigB", name="ctxT_sb")
        att_pool = es.enter_context(tc.tile_pool(name="attp", bufs=4))
        qk_pool = es.enter_context(tc.tile_pool(name="qkp", bufs=2))
        vb_pool = es.enter_context(tc.tile_pool(name="vbp", bufs=2))
        rr_pool = es.enter_context(tc.tile_pool(name="rrp", bufs=2))
        SKT = S // 128   # 16 kk tiles per batch
        for b in range(B):
            # v for this batch: [128, src(4), tt(4), h(2), 65] fp16 (65th col = 1)
            vb = vb_pool.tile([128, 4, TT, HPC, D + 1], F16, tag="vb")
            for i in range(4):
                src = 4 * b + i
                nc.sync.dma_start(
                    out=vb[:, i, :, :, 0:D],
                    in_=kv_out.ap()[src, 1].rearrange("(p tt h d) -> p tt h d",
                                                       p=128, tt=TT, h=HPC))
            nc.vector.memset(vb[:, :, :, :, D:D + 1], 1.0)
            for h in range(HPC):
                qa = qk_pool.tile([64, 4, T], F16, tag="qa")
                ka = qk_pool.tile([64, 4, T], F16, tag="ka")
                for i in range(4):
                    src = 4 * b + i
                    nc.sync.dma_start(
                        out=qa[:, i, :],
                        in_=q_out.ap()[src].rearrange("(p t) -> p t", p=128)[bass.ts(h, 64), :])
                    nc.sync.dma_start(
                        out=ka[:, i, :],
                        in_=kv_out.ap()[src, 0].rearrange("(p t) -> p t", p=128)[bass.ts(h, 64), :])
                qf = qa[:].rearrange("p a t -> p (a t)")
                kf = ka[:].rearrange("p a t -> p (a t)")
                for qp in range(S // 256):
                    nkt = 2 * qp + 2
                    cx = pmix.tile([65, 256], F32, tag="pmix", name="cx")
                    for kt in range(nkt):
                        if kt % 2 == 0:
                            sc = pacc.tile([128, 512], F32, tag=f"acc{kt % 4}", name="sc")
                            sc = sc[:, :256]
                        else:
                            sc = pmix.tile([128, 256], F32, tag="pmix", name="sc")
                        att = att_pool.tile([128, 256], F16, tag="att")
                        nc.tensor.matmul(sc[:], kf[:, bass.ts(kt, 128)],
                                         qf[:, bass.ts(qp, 256)], start=True, stop=True)
                        if kt < 2 * qp:
                            nc.any.tensor_scalar_max(att[:], sc[:], 0.0)
                        else:  # diagonal tiles: mask then relu
                            nc.any.tensor_mul(att[:], sc[:], tri[:, kt - 2 * qp, :])
                            nc.any.tensor_scalar_max(att[:], att[:], 0.0)
                        nc.tensor.matmul(cx[:], vb[:, kt // TT, kt % TT, h, :],
                                         att[:], start=(kt == 0), stop=(kt == nkt - 1))
                    # normalize: ctxT[d, q] * 1/(rowsum[q] + 1e-9)
                    rs = rr_pool.tile([1, 256], F32, tag="rs")
                    rb = rr_pool.tile([64, 256], F32, tag="rb")
                    nc.vector.tensor_scalar_add(rs[:], cx[64:65, :], 1e-9)
                    nc.vector.reciprocal(rs[:], rs[:])
                    rbp = pmix.tile([64, 256], F32, tag="pmix", name="rbp")
                    nc.tensor.matmul(rbp[:], ones64[:1, :], rs[:1, :], start=True, stop=True)
                    nc.vector.tensor_copy(rb[:], rbp[:])
                    nc.vector.tensor_tensor(
                        ctxT_sb[bass.ts(h, 64), b, bass.ts(qp, 256)],
                        cx[0:64, :], rb[:], MUL)

        # =====================================================================
        # A2A #2: head-sharded ctx -> token-sharded
        # =====================================================================
        for j in range(NC):
            nc.sync.dma_start(out=cc_in.ap()[j].rearrange("(p t) -> p t", p=128),
                              in_=ctxT_sb[:, :, :].rearrange("p b s -> p (b s)")[:, bass.ds(j * T, T)])
        nc.gpsimd.collective_compute(
            "AllToAll", mybir.AluOpType.bypass, replica_groups=RG,
            ins=[cc_in.ap().opt()], outs=[cc_out.ap().opt()])
        ctxo_sb = persist.tile([128, KT, T], F16, tag="bigC", name="ctxo_sb")
        for j in range(NC):
            nc.sync.dma_start(out=ctxo_sb[:, j, :],
                              in_=cc_out.ap()[j].rearrange("(p t) -> p t", p=128))
        ctxo32 = persist.tile([128, KT, T], F32R, tag="bigB", name="ctxo32")
        for j in range(NC):
            nc.any.tensor_copy(ctxo32[:, j, :], ctxo_sb[:, j, :])

        # =====================================================================
        # out-proj: x += ctx @ wo.T + bo  (fp16)
        # =====================================================================
        bcast_row(bias_bc, b_io["bo"], H)
        for nch in range(2):
            ps = acc_tiles()
            for kt in range(KT):
                wt = wpool.tile([128, 512], F32R, tag="wa")
                nc.sync.dma_start(out=wt[:], in_=_r(wT_io["woT"].ap()[bass.ts(kt, 128), bass.ts(nch, 512)]))
                for tt in range(TT):
                    nc.tensor.matmul(ps[tt][:], ctxo32[:, kt, bass.ts(tt, 128)],
                                     wt[:], start=(kt == 0), stop=(kt == KT - 1))
            for tt in range(TT):
                xsl = x_sb[:, tt, bass.ts(nch, 512)]
                tmp = sc_pool.tile([128, 512], F32, tag="ev")
                nc.any.tensor_add(tmp[:], ps[tt][:], bias_bc[:, bass.ts(nch, 512)])
                nc.any.tensor_add(xsl, xsl, tmp[:])

        # =====================================================================
        # FFN (fp16): x += relu(LN3(x) @ w1.T + b1f) @ w2.T + b2f
        # =====================================================================
        ln3T_sb = persist.tile([128, KT, T], F32R, tag="bigD", name="ln3T_sb")
        layer_norm_t("g3", "b3", ln3T_sb)
        h_sb = persist.tile([128, FF // 128, T], F16, tag="bigA", name="h_sb")
        NFT = FF // 128  # 32
        for nh in range(NFT // 4):
            ps = acc_tiles()
            for kt in range(KT):
                wt = wpool.tile([128, 512], F32R, tag="wa")
                nc.sync.dma_start(out=wt[:], in_=_r(w1T_io.ap()[bass.ts(kt, 128), bass.ts(nh, 512)]))
                for n4 in range(4):
                    nc.tensor.matmul(ps[n4][:], wt[:, bass.ts(n4, 128)], ln3T_sb[:, kt, :],
                                     start=(kt == 0), stop=(kt == KT - 1))
            for n4 in range(4):
                nt = nh * 4 + n4
                nc.scalar.activation(h_sb[:, nt, :], ps[n4][:], AF.Relu,
                                     bias=ff1b_col[:, nt:nt + 1])
        bcast_row(bias_bc, b_io["ff2_b"], H)
        for nch in range(2):
            ps = acc_tiles()
            for kt in range(NFT):
                wf = wpool.tile([128, 512], F32, tag="wb")
                nc.sync.dma_start(out=wf[:], in_=w2T_io.ap()[bass.ts(kt, 128), bass.ts(nch, 512)])
                wt = wpool16.tile([128, 512], F16, tag="w16")
                nc.any.tensor_copy(wt[:], wf[:])
                for tt in range(TT):
                    nc.tensor.matmul(ps[tt][:], h_sb[:, kt, bass.ts(tt, 128)],
                                     wt[:], start=(kt == 0), stop=(kt == NFT - 1))
            for tt in range(TT):
                xsl = x_sb[:, tt, bass.ts(nch, 512)]
                tmp = sc_pool.tile([128, 512], F32, tag="ev")
                nc.any.tensor_add(tmp[:], ps[tt][:], bias_bc[:, bass.ts(nch, 512)])
                nc.any.tensor_add(xsl, xsl, tmp[:])

        # final output
        nc.sync.dma_start(out=out_io.ap().rearrange("(tt p) h -> p tt h", p=128),
                          in_=x_sb[:])

    nc.compile()
    return nc


def _prep_shared(inputs):
    f = lambda a: np.ascontiguousarray(np.asarray(a, np.float32))
    sh = {
        "sfwT": f(inputs["sf_w"]).T.copy(),
        "maskT": f(inputs["mask"]).T.copy(),
        "wqT": f(inputs["wq"]).T.copy(),
        "wkT": f(inputs["wk"]).T.copy(),
        "wvT": f(inputs["wv"]).T.copy(),
        "woT": f(inputs["wo"]).T.copy(),
        "w1T": f(inputs["ff1_w"]).T.copy(),
        "w2T": f(inputs["ff2_w"]).T.copy(),
        "ff1_b": f(inputs["ff1_b"]),
    }
    for k in ("sf_b", "bq", "bk", "bv", "bo"):
        sh[k] = f(inputs[k])
    sh["ff2_b"] = f(inputs["ff2_b"])
    for k in ("g1", "b1", "g2", "b2", "g3", "b3"):
        sh[k] = f(inputs[k])
    # diag masks: tri[0] = [tril.T | ones], tri[1] = [zeros | tril.T]
    tri = np.zeros((2, 128, 256), np.float32)
    tl = np.tril(np.ones((128, 128), np.float32)).T  # valid: kk(row) <= q(col)
    tri[0, :, :128] = tl
    tri[0, :, 128:] = 1.0
    tri[1, :, 128:] = tl
    sh["tri"] = tri
    sh["bqk_col"] = np.stack([sh["bq"], sh["bk"]]).reshape(2 * KT, 128).T.copy().reshape(128, 2 * KT)
    sh["ff1b_col"] = sh["ff1_b"].reshape(FF // 128, 128).T.copy()
    return sh


def kernel(**inputs) -> np.ndarray:
    from concourse.bass_utils import run_bass_kernel_spmd

    if "nc" not in _CACHE:
        _CACHE["nc"] = _build()
    nc = _CACHE["nc"]

    sh = _prep_shared(inputs)
    x = np.ascontiguousarray(np.asarray(inputs["x"], np.float32)).reshape(B * S, H)
    in_maps = []
    for c in range(NC):
        m = dict(sh)
        m["x_c"] = np.ascontiguousarray(x[c * T:(c + 1) * T])
        in_maps.append(m)

    res = run_bass_kernel_spmd(nc, in_maps, core_ids=list(range(NC)))
    out = np.concatenate([res.results[c]["out_c"] for c in range(NC)], axis=0)
    return out.reshape(B, S, H).astype(np.float32)



# revision 9
# speedup vs baseline: 1.4468x; 1.4468x over previous
# Trainium2 Bass kernel for nn_BDHBlock (dense transformer block), v2.
#
# Strategy (8 NeuronCores, one shared SPMD program):
#   - Token-parallel for token-local stages with *batch-interleaved*
#     sharding: core c owns tokens b0[256c:256c+256] + b1[256c:256c+256].
#     This lets the attention AllToAlls split per batch so they overlap
#     with projection/attention compute.
#   - Attention is head-parallel (2 heads x full sequence per core) via
#     per-batch AllToAlls. Scores for the two heads run concurrently in
#     the PE array (K=64 row-tiling at base partitions 0/64).
#   - All weights are cast to fp16 on the host (halves HBM traffic);
#     the sparsity mask is folded into sf_w on the host.
#   - Biases enter PSUM via rank-1 (K=1) matmuls; layernorm gamma/beta
#     are applied per-partition during the transpose evacuation.
import numpy as np

import concourse.bass as bass
import concourse.mybir as mybir
import concourse.tile as tile
from concourse import bacc
from concourse.masks import make_identity

B, S, H, NH = 2, 2048, 1024, 16
D = H // NH            # 64
FF = 4 * H             # 4096
NC = 8                 # cores
T = B * S // NC        # 512 tokens per core (256 per batch)
TPB = T // B           # 256 tokens per batch per core
TT = T // 128          # 4 token tiles
KT = H // 128          # 8 feature tiles
NFT = FF // 128        # 32 hidden tiles
HPC = 2                # heads per core
F32, F32R, F16 = mybir.dt.float32, mybir.dt.float32r, mybir.dt.float16
ADD, SUB, MUL, MAX = (mybir.AluOpType.add, mybir.AluOpType.subtract,
                      mybir.AluOpType.mult, mybir.AluOpType.max)
AF = mybir.ActivationFunctionType
RG = [list(range(NC))]
EPS = 1e-5
QSC = 1.0 / float(np.sqrt(np.sqrt(D)))
SLOT = 128 * TPB       # elements per (dest, tensor) A2A slot

_CACHE = {}


def _r(ap):
    return ap.bitcast(F32R)


def _build():
    nc = bacc.Bacc("TRN2", target_bir_lowering=False, debug=False,
                   num_devices=NC)

    # ---------------- I/O ----------------
    def inp(name, shape, dtype):
        return nc.dram_tensor(name, list(shape), dtype, kind="ExternalInput")

    x_io = inp("x_c", (T, H), F32)
    sfwmT_io = inp("sfwmT", (H, H), F16)
    wqT_io = inp("wqT", (H, H), F16)
    wkT_io = inp("wkT", (H, H), F16)
    wvT_io = inp("wvT", (H, H), F16)
    woT_io = inp("woT", (H, H), F16)
    w1T_io = inp("w1T", (H, FF), F16)
    w2T_io = inp("w2T", (FF, H), F16)
    bias_io = inp("bias_rows", (1, 4 * H), F16)   # sf_b | bv | bo | ff2_b
    bqk_io = inp("bqk_col", (128, 2 * KT), F32)   # (b+..)*qsc pre-scaled
    ff1b_io = inp("ff1b_col", (128, NFT), F32)
    gb_io = inp("gb_cols", (128, 6 * KT), F32)    # g1 b1 g2 b2 g3 b3
    tri_io = inp("tri4", (4, 128, 512), F16)      # causal diag masks
    out_io = nc.dram_tensor("out_c", [T, H], F32, kind="ExternalOutput")

    # internal DRAM for collectives (HBM bounce)
    kvq_in = [nc.dram_tensor(f"kvq_in{b}", [NC, 3, SLOT], F16) for b in range(B)]
    kvq_out = [nc.dram_tensor(f"kvq_out{b}", [NC, 3, SLOT], F16) for b in range(B)]
    cc_in = [nc.dram_tensor(f"cc_in{b}", [NC, SLOT], F16) for b in range(B)]
    cc_out = [nc.dram_tensor(f"cc_out{b}", [NC, SLOT], F16) for b in range(B)]

    from contextlib import ExitStack
    with tile.TileContext(nc) as tc, ExitStack() as es:
        # ---------------- pools ----------------
        const = es.enter_context(tc.tile_pool(name="const", bufs=1))
        persist = es.enter_context(tc.tile_pool(name="persist", bufs=1))
        wpool = es.enter_context(tc.tile_pool(name="wpool", bufs=6))
        sc_pool = es.enter_context(tc.tile_pool(name="scratch", bufs=2))
        small = es.enter_context(tc.tile_pool(name="small", bufs=8))
        rs_pool = es.enter_context(tc.tile_pool(name="rsp", bufs=2))
        att_in = es.enter_context(tc.tile_pool(name="attin", bufs=2))
        attb = es.enter_context(tc.tile_pool(name="attb", bufs=6))
        rr_pool = es.enter_context(tc.tile_pool(name="rrp", bufs=1))
        pacc = es.enter_context(tc.tile_pool(name="pacc", bufs=1, space="PSUM"))
        pmix = es.enter_context(tc.tile_pool(name="pmix", bufs=4, space="PSUM"))

        # ---------------- constants (gpsimd DMA queue) ----------------
        ident = const.tile([128, 128], F32)
        make_identity(nc, ident)
        tri = const.tile([128, 4, 512], F16)
        nc.gpsimd.dma_start(out=tri[:], in_=tri_io.ap().rearrange("a p q -> p a q"))
        bqk_col = const.tile([128, 2 * KT], F32)
        nc.gpsimd.dma_start(out=bqk_col[:], in_=bqk_io.ap())
        ff1b_col = const.tile([128, NFT], F32)
        nc.gpsimd.dma_start(out=ff1b_col[:], in_=ff1b_io.ap())
        gb_cols = const.tile([128, 6 * KT], F32)
        nc.gpsimd.dma_start(out=gb_cols[:], in_=gb_io.ap())
        bias_sb = const.tile([1, 4 * H], F16)
        nc.gpsimd.dma_start(out=bias_sb[:], in_=bias_io.ap())
        ones1 = const.tile([1, 128], F16)
        nc.vector.memset(ones1[:], 1.0)
        ones64 = const.tile([1, 64], F16)
        nc.vector.memset(ones64[:], 1.0)
        eps_col = const.tile([128, 1], F32)
        nc.vector.memset(eps_col[:], EPS)

        # q/k/v/o weights resident in SBUF (gpsimd queue, overlaps LN1/stage1)
        wB = persist.tile([128, 4, KT, H], F16, tag="wB")
        for wi, wio in enumerate((wqT_io, wkT_io, wvT_io, woT_io)):
            nc.gpsimd.dma_start(
                out=wB[:, wi], in_=wio.ap().rearrange("(kt p) h -> p kt h", p=128))

        # residual stream, token-major [128, tt, H] f32
        x_sb = persist.tile([128, TT, H], F32)
        for tt in range(TT):
            nc.sync.dma_start(
                out=x_sb[:, tt, :],
                in_=x_io.ap().rearrange("(tt p) h -> p tt h", p=128)[:, tt])

        lnT = persist.tile([128, KT, T], F16, tag="lnT")
        qT = persist.tile([128, KT, T], F16, tag="qT")
        kT = persist.tile([128, KT, T], F16, tag="kT")
        v_sb = persist.tile([128, TT, H], F16, tag="v")
        ctxT = persist.tile([128, B, S], F16, tag="ctxT")
        ctxo = persist.tile([128, KT, T], F16, tag="ctxo")

        _round = [0]

        def acc_tiles(n=4, cols=512):
            r = _round[0]
            _round[0] += 1
            if r % 2 == 0:
                return [pacc.tile([128, cols], F32, tag=f"acc{t}", name=f"acc{t}")
                        for t in range(n)]
            return [pmix.tile([128, cols], F32, tag="pmix", name=f"accp{t}")
                    for t in range(n)]

        # ---------------- layernorm (token-major) + transpose ----------------
        def layer_norm_t(gb_base, dst, tts):
            for tt in tts:
                xt = x_sb[:, tt, :]
                sums = small.tile([128, 1], F32, tag="s0")
                sumsq = small.tile([128, 1], F32, tag="s1")
                sq = sc_pool.tile([128, H], F32, tag="lnt", name="lnsq")
                nc.vector.reduce_sum(sums[:], xt, axis=mybir.AxisListType.X)
                nc.scalar.activation(sq[:], xt, AF.Square, accum_out=sumsq[:])
                mu = small.tile([128, 1], F32, tag="s2")
                var = small.tile([128, 1], F32, tag="s3")
                rstd = small.tile([128, 1], F32, tag="s4")
                nc.vector.tensor_scalar_mul(mu[:], sums[:], 1.0 / H)
                nc.vector.tensor_scalar_mul(var[:], sumsq[:], 1.0 / H)
                nc.vector.tensor_tensor(rstd[:], mu[:], mu[:], MUL)
                nc.vector.tensor_tensor(var[:], var[:], rstd[:], SUB)
                nc.scalar.activation(rstd[:], var[:], AF.Sqrt, bias=eps_col[:])
                nc.vector.reciprocal(rstd[:], rstd[:])
                lt = sc_pool.tile([128, H], F32, tag="lnt")
                nc.vector.tensor_scalar(lt[:], xt, mu[:], rstd[:], op0=SUB, op1=MUL)
                for kt in range(KT):
                    pt = pmix.tile([128, 128], F32, tag="pmix", name="pt")
                    nc.tensor.transpose(pt[:], lt[:, bass.ts(kt, 128)], ident[:])
                    g_ap = gb_cols[:, gb_base + kt: gb_base + kt + 1]
                    b_ap = gb_cols[:, gb_base + KT + kt: gb_base + KT + kt + 1]
                    if kt % 2 == 0:
                        nc.vector.tensor_scalar(dst[:, kt, bass.ts(tt, 128)], pt[:],
                                                g_ap, b_ap, op0=MUL, op1=ADD)
                    else:
                        nc.scalar.activation(dst[:, kt, bass.ts(tt, 128)], pt[:],
                                             AF.Identity, scale=g_ap, bias=b_ap)

        # =====================================================================
        # Stage 1: x += LN1(x) @ (sf_w * mask).T + sf_b
        # =====================================================================
        layer_norm_t(0, lnT, range(TT))
        for nch in range(2):
            ps = acc_tiles()
            for tt in range(TT):
                nc.tensor.matmul(ps[tt][:], ones1[:],
                                 bias_sb[:, bass.ds(nch * 512, 512)],
                                 start=True, stop=False)
            for kt in range(KT):
                wt = wpool.tile([128, 512], F16, tag="wa")
                nc.sync.dma_start(
                    out=wt[:],
                    in_=sfwmT_io.ap()[bass.ts(kt, 128), bass.ts(nch, 512)])
                for tt in range(TT):
                    nc.tensor.matmul(ps[tt][:], lnT[:, kt, bass.ts(tt, 128)],
                                     wt[:], start=False, stop=(kt == KT - 1))
            for tt in range(TT):
                xsl = x_sb[:, tt, bass.ts(nch, 512)]
                nc.vector.tensor_tensor(xsl, xsl, ps[tt][:], ADD)

        # =====================================================================
        # Stage 2: LN2 + QKV (split per batch so A2A b0 launches early)
        # =====================================================================
        def proj_qk(half):
            for wi, dst, cb in ((0, qT, 0), (1, kT, KT)):
                for nh in range(2):
                    ps = acc_tiles(cols=TPB)
                    for kt in range(KT):
                        for n4 in range(4):
                            nc.tensor.matmul(
                                ps[n4][:],
                                wB[:, wi, kt, bass.ds(nh * 512 + n4 * 128, 128)],
                                lnT[:, kt, bass.ds(half * TPB, TPB)],
                                start=(kt == 0), stop=(kt == KT - 1))
                    for n4 in range(4):
                        nt = nh * 4 + n4
                        col = bqk_col[:, cb + nt: cb + nt + 1]
                        dsl = dst[:, nt, bass.ds(half * TPB, TPB)]
                        if n4 % 2 == 0:
                            nc.vector.tensor_scalar(dsl, ps[n4][:], QSC, col,
                                                    op0=MUL, op1=ADD)
                        else:
                            nc.scalar.activation(dsl, ps[n4][:], AF.Identity,
                                                 scale=QSC, bias=col)

        def proj_v(half):
            for tt in (2 * half, 2 * half + 1):
                ps = acc_tiles(n=2)
                for nch in range(2):
                    nc.tensor.matmul(ps[nch][:], ones1[:],
                                     bias_sb[:, bass.ds(H + nch * 512, 512)],
                                     start=True, stop=False)
                for kt in range(KT):
                    for nch in range(2):
                        nc.tensor.matmul(ps[nch][:], lnT[:, kt, bass.ts(tt, 128)],
                                         wB[:, 2, kt, bass.ts(nch, 512)],
                                         start=False, stop=(kt == KT - 1))
                for nch in range(2):
                    dsl = v_sb[:, tt, bass.ts(nch, 512)]
                    if nch == 0:
                        nc.vector.tensor_copy(dsl, ps[nch][:])
                    else:
                        nc.scalar.activation(dsl, ps[nch][:], AF.Copy)

        def pack_kvq(b):
            h0 = b * TPB
            for j in range(NC):
                nc.sync.dma_start(
                    out=kvq_in[b].ap()[j, 0].rearrange("(p t) -> p t", p=128),
                    in_=kT[:, j, bass.ds(h0, TPB)])
                nc.sync.dma_start(
                    out=kvq_in[b].ap()[j, 1].rearrange(
                        "(p tt f) -> p tt f", p=128, tt=2),
                    in_=v_sb[:, 2 * b:2 * b + 2, bass.ts(j, 128)])
                nc.sync.dma_start(
                    out=kvq_in[b].ap()[j, 2].rearrange("(p t) -> p t", p=128),
                    in_=qT[:, j, bass.ds(h0, TPB)])
            nc.gpsimd.collective_compute(
                "AllToAll", mybir.AluOpType.bypass, replica_groups=RG,
                ins=[kvq_in[b].ap().opt()], outs=[kvq_out[b].ap().opt()])

        layer_norm_t(2 * KT, lnT, (0, 1))
        proj_qk(0)
        proj_v(0)
        pack_kvq(0)
        layer_norm_t(2 * KT, lnT, (2, 3))
        proj_qk(1)
        proj_v(1)
        pack_kvq(1)

        # =====================================================================
        # Attention: 2 heads, full sequence, exact causal, per batch
        # =====================================================================
        def attn_assemble(b):
            qa = att_in.tile([128, S], F16, tag="qa")
            ka = att_in.tile([128, S], F16, tag="ka")
            vb = att_in.tile([128, S // 128, HPC, D + 1], F16, tag="vb")
            for i in range(NC):
                nc.sync.dma_start(
                    out=qa[:, bass.ts(i, TPB)],
                    in_=kvq_out[b].ap()[i, 2].rearrange("(p t) -> p t", p=128))
                nc.sync.dma_start(
                    out=ka[:, bass.ts(i, TPB)],
                    in_=kvq_out[b].ap()[i, 0].rearrange("(p t) -> p t", p=128))
                nc.sync.dma_start(
                    out=vb[:, 2 * i:2 * i + 2, :, 0:D],
                    in_=kvq_out[b].ap()[i, 1].rearrange(
                        "(p tt h d) -> p tt h d", p=128, tt=2, h=HPC))
            nc.vector.memset(vb[:, :, :, D:D + 1], 1.0)
            return qa, ka, vb

        def attn_compute(b, qa, ka, vb):
            for qp in range(S // 512):
                nkt = 4 * (qp + 1)
                cx = [pacc.tile([D + 1, 512], F32, tag=f"acc{2 * h + qp % 2}",
                                name=f"cx{h}") for h in range(HPC)]

                def scores(kt):
                    out = []
                    for h in range(HPC):
                        sc = pmix.tile([128, 512], F32, tag="pmix", name="sc")
                        nc.tensor.matmul(
                            sc[:], ka[bass.ts(h, 64), bass.ts(kt, 128)],
                            qa[bass.ts(h, 64), bass.ts(qp, 512)],
                            start=True, stop=True)
                        out.append(sc)
                    return out

                sc_cur = scores(0)
                for kt in range(nkt):
                    sc_nxt = scores(kt + 1) if kt + 1 < nkt else None
                    atts = []
                    for h in range(HPC):
                        att = attb.tile([128, 512], F16, tag="att")
                        if kt >= 4 * qp:  # diagonal tile: relu then mask
                            nc.scalar.activation(att[:], sc_cur[h][:], AF.Relu)
                            nc.vector.tensor_tensor(
                                att[:], att[:], tri[:, kt - 4 * qp, :], MUL)
                        elif (kt + h) % 2 == 0:
                            nc.vector.tensor_scalar_max(att[:], sc_cur[h][:], 0.0)
                        else:
                            nc.scalar.activation(att[:], sc_cur[h][:], AF.Relu)
                        atts.append(att)
                    for h in range(HPC):
                        nc.tensor.matmul(cx[h][:], vb[:, kt, h, :], atts[h][:],
                                         start=(kt == 0), stop=(kt == nkt - 1))
                    sc_cur = sc_nxt
                for h in range(HPC):
                    rs = rs_pool.tile([1, 512], F16, tag=f"rs{h}")
                    # eps must survive the f16 cast (f16 min normal ~6.1e-5):
                    # an all-masked row then yields 0 * recip(eps) = 0, not NaN.
                    nc.vector.tensor_scalar_add(rs[:], cx[h][D:D + 1, :], 6.5e-5)
                    with nc.allow_low_precision(reason="attn rowsum recip f16"):
                        nc.vector.reciprocal(rs[:], rs[:])
                    rbp = pmix.tile([D, 512], F32, tag="pmix", name="rbp")
                    nc.tensor.matmul(rbp[:], ones64[:], rs[:],
                                     start=True, stop=True)
                    rb = rr_pool.tile([D, 512], F32, tag=f"rb{h}")
                    nc.vector.tensor_copy(rb[:], rbp[:])
                    nc.vector.tensor_tensor(
                        ctxT[bass.ts(h, 64), b, bass.ts(qp, 512)],
                        cx[h][0:D, :], rb[:], MUL)

        def pack_cc(b):
            for j in range(NC):
                nc.sync.dma_start(
                    out=cc_in[b].ap()[j].rearrange("(p t) -> p t", p=128),
                    in_=ctxT[:, b, bass.ts(j, TPB)])
            nc.gpsimd.collective_compute(
                "AllToAll", mybir.AluOpType.bypass, replica_groups=RG,
                ins=[cc_in[b].ap().opt()], outs=[cc_out[b].ap().opt()])

        def unpack_cc(b):
            for i in range(NC):
                nc.sync.dma_start(
                    out=ctxo[:, i, bass.ds(b * TPB, TPB)],
                    in_=cc_out[b].ap()[i].rearrange("(p t) -> p t", p=128))

        a0 = attn_assemble(0)
        attn_compute(0, *a0)
        a1 = attn_assemble(1)
        pack_cc(0)
        attn_compute(1, *a1)
        pack_cc(1)
        unpack_cc(0)
        unpack_cc(1)

        # =====================================================================
        # out-proj: x += ctx @ wo.T + bo
        # =====================================================================
        for half in range(2):
            for nch in range(2):
                ps = acc_tiles(n=2)
                for i, tt in enumerate((2 * half, 2 * half + 1)):
                    nc.tensor.matmul(ps[i][:], ones1[:],
                                     bias_sb[:, bass.ds(2 * H + nch * 512, 512)],
                                     start=True, stop=False)
                for kt in range(KT):
                    for i, tt in enumerate((2 * half, 2 * half + 1)):
                        nc.tensor.matmul(ps[i][:], ctxo[:, kt, bass.ts(tt, 128)],
                                         wB[:, 3, kt, bass.ts(nch, 512)],
                                         start=False, stop=(kt == KT - 1))
                for i, tt in enumerate((2 * half, 2 * half + 1)):
                    xsl = x_sb[:, tt, bass.ts(nch, 512)]
                    nc.vector.tensor_tensor(xsl, xsl, ps[i][:], ADD)

        # =====================================================================
        # FFN: x += relu(LN3(x) @ w1.T + b1f) @ w2.T + b2f
        # =====================================================================
        layer_norm_t(4 * KT, lnT, range(TT))
        h_sb = persist.tile([128, NFT, T], F16, tag="wB", name="h_sb")
        for nh in range(NFT // 4):
            ps = acc_tiles()
            for kt in range(KT):
                wt = wpool.tile([128, 512], F16, tag="wa")
                nc.sync.dma_start(
                    out=wt[:], in_=w1T_io.ap()[bass.ts(kt, 128), bass.ts(nh, 512)])
                for n4 in range(4):
                    nc.tensor.matmul(ps[n4][:], wt[:, bass.ts(n4, 128)],
                                     lnT[:, kt, :],
                                     start=(kt == 0), stop=(kt == KT - 1))
            for n4 in range(4):
                nt = nh * 4 + n4
                if n4 % 2 == 0:
                    nc.scalar.activation(h_sb[:, nt, :], ps[n4][:], AF.Relu,
                                         bias=ff1b_col[:, nt:nt + 1])
                else:
                    nc.vector.tensor_scalar(h_sb[:, nt, :], ps[n4][:],
                                            ff1b_col[:, nt:nt + 1], 0.0,
                                            op0=ADD, op1=MAX)
        for nch in range(2):
            ps = acc_tiles()
            for tt in range(TT):
                nc.tensor.matmul(ps[tt][:], ones1[:],
                                 bias_sb[:, bass.ds(3 * H + nch * 512, 512)],
                                 start=True, stop=False)
            for kt in range(NFT):
                wt = wpool.tile([128, 512], F16, tag="wa")
                nc.sync.dma_start(
                    out=wt[:], in_=w2T_io.ap()[bass.ts(kt, 128), bass.ts(nch, 512)])
                for tt in range(TT):
                    nc.tensor.matmul(ps[tt][:], h_sb[:, kt, bass.ts(tt, 128)],
                                     wt[:], start=False, stop=(kt == NFT - 1))
            for tt in range(TT):
                xsl = x_sb[:, tt, bass.ts(nch, 512)]
                nc.vector.tensor_tensor(xsl, xsl, ps[tt][:], ADD)

        # final output
        nc.sync.dma_start(out=out_io.ap().rearrange("(tt p) h -> p tt h", p=128),
                          in_=x_sb[:])

    nc.compile()
    return nc


def _prep_shared(inputs):
    f = lambda a: np.asarray(a, np.float32)
    h = lambda a: np.ascontiguousarray(a.astype(np.float16))
    sh = {
        "sfwmT": h((f(inputs["sf_w"]) * f(inputs["mask"])).T),
        "wqT": h(f(inputs["wq"]).T),
        "wkT": h(f(inputs["wk"]).T),
        "wvT": h(f(inputs["wv"]).T),
        "woT": h(f(inputs["wo"]).T),
        "w1T": h(f(inputs["ff1_w"]).T),
        "w2T": h(f(inputs["ff2_w"]).T),
    }
    sh["bias_rows"] = h(np.concatenate(
        [f(inputs["sf_b"]), f(inputs["bv"]), f(inputs["bo"]),
         f(inputs["ff2_b"])]).reshape(1, 4 * H))
    bqk = np.stack([f(inputs["bq"]), f(inputs["bk"])]) * QSC
    sh["bqk_col"] = np.ascontiguousarray(bqk.reshape(2 * KT, 128).T)
    sh["ff1b_col"] = np.ascontiguousarray(
        f(inputs["ff1_b"]).reshape(NFT, 128).T)
    gb = np.concatenate([f(inputs[k]) for k in
                         ("g1", "b1", "g2", "b2", "g3", "b3")])
    sh["gb_cols"] = np.ascontiguousarray(gb.reshape(6 * KT, 128).T)
    tri = np.zeros((4, 128, 512), np.float16)
    for d in range(4):
        for p in range(128):
            tri[d, p, 128 * d + p:] = 1.0
    sh["tri4"] = tri
    return sh


def make_in_maps(inputs):
    sh = _prep_shared(inputs)
    x = np.asarray(inputs["x"], np.float32).reshape(B, NC, TPB, H)
    in_maps = []
    for c in range(NC):
        m = dict(sh)
        m["x_c"] = np.ascontiguousarray(
            np.concatenate([x[0, c], x[1, c]], axis=0))
        in_maps.append(m)
    return in_maps


def assemble_out(results):
    out = np.empty((B, S, H), np.float32)
    for c in range(NC):
        r = results[c]["out_c"]
        out[0, c * TPB:(c + 1) * TPB] = r[:TPB]
        out[1, c * TPB:(c + 1) * TPB] = r[TPB:]
    return out


def kernel(**inputs) -> np.ndarray:
    from concourse.bass_utils import run_bass_kernel_spmd

    if "nc" not in _CACHE:
        _CACHE["nc"] = _build()
    nc = _CACHE["nc"]

    in_maps = make_in_maps(inputs)
    res = run_bass_kernel_spmd(nc, in_maps, core_ids=list(range(NC)))
    return assemble_out(res.results)


# revision 26
# speedup vs baseline: 1.4755x; 1.0199x over previous
# Trainium2 Bass kernel for nn_BDHBlock (dense transformer block), v2.
#
# Strategy (8 NeuronCores, one shared SPMD program):
#   - Token-parallel for token-local stages with *batch-interleaved*
#     sharding: core c owns tokens b0[256c:256c+256] + b1[256c:256c+256].
#     This lets the attention AllToAlls split per batch so they overlap
#     with projection/attention compute.
#   - Attention is head-parallel (2 heads x full sequence per core) via
#     per-batch AllToAlls. Scores for the two heads run concurrently in
#     the PE array (K=64 row-tiling at base partitions 0/64).
#   - All weights are cast to fp16 on the host (halves HBM traffic);
#     the sparsity mask is folded into sf_w on the host.
#   - Biases enter PSUM via rank-1 (K=1) matmuls; layernorm gamma/beta
#     are applied per-partition during the transpose evacuation.
import numpy as np

import concourse.bass as bass
import concourse.mybir as mybir
import concourse.tile as tile
from concourse import bacc
from concourse.masks import make_identity

B, S, H, NH = 2, 2048, 1024, 16
D = H // NH            # 64
FF = 4 * H             # 4096
NC = 8                 # cores
T = B * S // NC        # 512 tokens per core (256 per batch)
TPB = T // B           # 256 tokens per batch per core
TT = T // 128          # 4 token tiles
KT = H // 128          # 8 feature tiles
NFT = FF // 128        # 32 hidden tiles
HPC = 2                # heads per core
F32, F32R, F16 = mybir.dt.float32, mybir.dt.float32r, mybir.dt.float16
ADD, SUB, MUL, MAX = (mybir.AluOpType.add, mybir.AluOpType.subtract,
                      mybir.AluOpType.mult, mybir.AluOpType.max)
AF = mybir.ActivationFunctionType
RG = [list(range(NC))]
EPS = 1e-5
QSC = 1.0 / float(np.sqrt(np.sqrt(D)))
SLOT = 128 * TPB       # elements per (dest, tensor) A2A slot

_CACHE = {}


def _r(ap):
    return ap.bitcast(F32R)


def _build():
    nc = bacc.Bacc("TRN2", target_bir_lowering=False, debug=False,
                   num_devices=NC)

    # ---------------- I/O ----------------
    def inp(name, shape, dtype):
        return nc.dram_tensor(name, list(shape), dtype, kind="ExternalInput")

    x_io = inp("x_c", (T, H), F32)
    sfwmT_io = inp("sfwmT", (H, H), F16)
    wqT_io = inp("wqT", (H, H), F16)
    wkT_io = inp("wkT", (H, H), F16)
    wvT_io = inp("wvT", (H, H), F16)
    woT_io = inp("woT", (H, H), F16)
    w1T_io = inp("w1T", (H, FF), F16)
    w2T_io = inp("w2T", (FF, H), F16)
    bias_io = inp("bias_rows", (1, 4 * H), F16)   # sf_b | bv | bo | ff2_b
    bqk_io = inp("bqk_col", (128, 2 * KT), F32)   # (b+..)*qsc pre-scaled
    ff1b_io = inp("ff1b_col", (128, NFT), F32)
    gb_io = inp("gb_cols", (128, 6 * KT), F32)    # g1 b1 g2 b2 g3 b3
    tri_io = inp("tri4", (4, 128, 512), F16)      # causal diag masks
    out_io = nc.dram_tensor("out_c", [T, H], F32, kind="ExternalOutput")

    # internal DRAM for collectives (HBM bounce)
    kvq_in = [nc.dram_tensor(f"kvq_in{b}", [NC, 3, SLOT], F16) for b in range(B)]
    kvq_out = [nc.dram_tensor(f"kvq_out{b}", [NC, 3, SLOT], F16) for b in range(B)]
    cc_in = [nc.dram_tensor(f"cc_in{b}", [NC, SLOT], F16) for b in range(B)]
    cc_out = [nc.dram_tensor(f"cc_out{b}", [NC, SLOT], F16) for b in range(B)]
    # bounce for broadcasting attention row-scale factors across partitions
    rows_dram = [nc.dram_tensor(f"rows{b}", [2 * (S // 512), 512], F16)
                 for b in range(B)]

    from contextlib import ExitStack
    with tile.TileContext(nc) as tc, ExitStack() as es:
        # ---------------- pools ----------------
        const = es.enter_context(tc.tile_pool(name="const", bufs=1))
        persist = es.enter_context(tc.tile_pool(name="persist", bufs=1))
        wpool = es.enter_context(tc.tile_pool(name="wpool", bufs=6))
        sc_pool = es.enter_context(tc.tile_pool(name="scratch", bufs=2))
        small = es.enter_context(tc.tile_pool(name="small", bufs=8))
        att_in = es.enter_context(tc.tile_pool(name="attin", bufs=2))
        attb = es.enter_context(tc.tile_pool(name="attb", bufs=6))
        norm_pool = es.enter_context(tc.tile_pool(name="normp", bufs=1))
        pacc = es.enter_context(tc.tile_pool(name="pacc", bufs=1, space="PSUM"))
        pmix = es.enter_context(tc.tile_pool(name="pmix", bufs=4, space="PSUM"))

        # ---------------- constants (gpsimd DMA queue) ----------------
        ident = const.tile([128, 128], F32)
        make_identity(nc, ident)
        tri = const.tile([128, 4, 512], F16)
        nc.gpsimd.dma_start(out=tri[:], in_=tri_io.ap().rearrange("a p q -> p a q"))
        bqk_col = const.tile([128, 2 * KT], F32)
        nc.gpsimd.dma_start(out=bqk_col[:], in_=bqk_io.ap())
        ff1b_col = const.tile([128, NFT], F32)
        nc.gpsimd.dma_start(out=ff1b_col[:], in_=ff1b_io.ap())
        gb_cols = const.tile([128, 6 * KT], F32)
        nc.gpsimd.dma_start(out=gb_cols[:], in_=gb_io.ap())
        bias_sb = const.tile([1, 4 * H], F16)
        nc.gpsimd.dma_start(out=bias_sb[:], in_=bias_io.ap())
        ones1 = const.tile([1, 128], F16)
        nc.vector.memset(ones1[:], 1.0)
        ones64 = const.tile([1, 64], F16)
        nc.vector.memset(ones64[:], 1.0)
        eps_col = const.tile([128, 1], F32)
        nc.vector.memset(eps_col[:], EPS)

        # q/k/v/o weights resident in SBUF (gpsimd queue, overlaps LN1/stage1)
        wB = persist.tile([128, 4, KT, H], F16, tag="wB")
        for wi, wio in enumerate((wqT_io, wkT_io, wvT_io, woT_io)):
            nc.gpsimd.dma_start(
                out=wB[:, wi], in_=wio.ap().rearrange("(kt p) h -> p kt h", p=128))

        # residual stream, token-major [128, tt, H] f32
        x_sb = persist.tile([128, TT, H], F32)
        for tt in range(TT):
            nc.sync.dma_start(
                out=x_sb[:, tt, :],
                in_=x_io.ap().rearrange("(tt p) h -> p tt h", p=128)[:, tt])

        lnT = persist.tile([128, KT, T], F16, tag="lnT")
        qT = persist.tile([128, KT, T], F16, tag="qT")
        kT = persist.tile([128, KT, T], F16, tag="kT")
        v_sb = persist.tile([128, TT, H], F16, tag="v")
        ctxT = persist.tile([128, B, S], F16, tag="ctxT")
        # qT is dead once both kvq packs are sent; reuse its space for ctxo
        ctxo = persist.tile([128, KT, T], F16, tag="qT", name="ctxo")

        _round = [0]

        def acc_tiles(n=4, cols=512):
            r = _round[0]
            _round[0] += 1
            if r % 2 == 0:
                return [pacc.tile([128, cols], F32, tag=f"acc{t}", name=f"acc{t}")
                        for t in range(n)]
            return [pmix.tile([128, cols], F32, tag="pmix", name=f"accp{t}")
                    for t in range(n)]

        # ---------------- layernorm (token-major) + transpose ----------------
        def layer_norm_t(gb_base, dst, tts):
            for tt in tts:
                xt = x_sb[:, tt, :]
                sums = small.tile([128, 1], F32, tag="s0")
                sumsq = small.tile([128, 1], F32, tag="s1")
                sq = sc_pool.tile([128, H], F32, tag="lnt", name="lnsq")
                nc.vector.reduce_sum(sums[:], xt, axis=mybir.AxisListType.X)
                nc.scalar.activation(sq[:], xt, AF.Square, accum_out=sumsq[:])
                mu = small.tile([128, 1], F32, tag="s2")
                var = small.tile([128, 1], F32, tag="s3")
                rstd = small.tile([128, 1], F32, tag="s4")
                nc.vector.tensor_scalar_mul(mu[:], sums[:], 1.0 / H)
                nc.vector.tensor_scalar_mul(var[:], sumsq[:], 1.0 / H)
                nc.vector.tensor_tensor(rstd[:], mu[:], mu[:], MUL)
                nc.vector.tensor_tensor(var[:], var[:], rstd[:], SUB)
                nc.scalar.activation(rstd[:], var[:], AF.Sqrt, bias=eps_col[:])
                nc.vector.reciprocal(rstd[:], rstd[:])
                lt = sc_pool.tile([128, H], F32, tag="lnt")
                nc.vector.tensor_scalar(lt[:], xt, mu[:], rstd[:], op0=SUB, op1=MUL)
                for kt in range(KT):
                    pt = pmix.tile([128, 128], F32, tag="pmix", name="pt")
                    nc.tensor.transpose(pt[:], lt[:, bass.ts(kt, 128)], ident[:])
                    g_ap = gb_cols[:, gb_base + kt: gb_base + kt + 1]
                    b_ap = gb_cols[:, gb_base + KT + kt: gb_base + KT + kt + 1]
                    if kt % 2 == 0:
                        nc.vector.tensor_scalar(dst[:, kt, bass.ts(tt, 128)], pt[:],
                                                g_ap, b_ap, op0=MUL, op1=ADD)
                    else:
                        nc.scalar.activation(dst[:, kt, bass.ts(tt, 128)], pt[:],
                                             AF.Identity, scale=g_ap, bias=b_ap)

        # =====================================================================
        # Stage 1: x += LN1(x) @ (sf_w * mask).T + sf_b
        # =====================================================================
        layer_norm_t(0, lnT, range(TT))
        for nch in range(2):
            ps = acc_tiles()
            for tt in range(TT):
                nc.tensor.matmul(ps[tt][:], ones1[:],
                                 bias_sb[:, bass.ds(nch * 512, 512)],
                                 start=True, stop=False)
            for kt in range(KT):
                wt = wpool.tile([128, 512], F16, tag="wa")
                nc.sync.dma_start(
                    out=wt[:],
                    in_=sfwmT_io.ap()[bass.ts(kt, 128), bass.ts(nch, 512)])
                for tt in range(TT):
                    nc.tensor.matmul(ps[tt][:], lnT[:, kt, bass.ts(tt, 128)],
                                     wt[:], start=False, stop=(kt == KT - 1))
            for tt in range(TT):
                xsl = x_sb[:, tt, bass.ts(nch, 512)]
                nc.vector.tensor_tensor(xsl, xsl, ps[tt][:], ADD)

        # =====================================================================
        # Stage 2: LN2 + QKV (split per batch so A2A b0 launches early)
        # =====================================================================
        def proj_qk(half):
            for wi, dst, cb in ((0, qT, 0), (1, kT, KT)):
                for nh in range(2):
                    ps = acc_tiles(cols=TPB)
                    for kt in range(KT):
                        for n4 in range(4):
                            nc.tensor.matmul(
                                ps[n4][:],
                                wB[:, wi, kt, bass.ds(nh * 512 + n4 * 128, 128)],
                                lnT[:, kt, bass.ds(half * TPB, TPB)],
                                start=(kt == 0), stop=(kt == KT - 1))
                    for n4 in range(4):
                        nt = nh * 4 + n4
                        col = bqk_col[:, cb + nt: cb + nt + 1]
                        dsl = dst[:, nt, bass.ds(half * TPB, TPB)]
                        if n4 % 2 == 0:
                            nc.vector.tensor_scalar(dsl, ps[n4][:], QSC, col,
                                                    op0=MUL, op1=ADD)
                        else:
                            nc.scalar.activation(dsl, ps[n4][:], AF.Identity,
                                                 scale=QSC, bias=col)

        def proj_v(half):
            for tt in (2 * half, 2 * half + 1):
                ps = acc_tiles(n=2)
                for nch in range(2):
                    nc.tensor.matmul(ps[nch][:], ones1[:],
                                     bias_sb[:, bass.ds(H + nch * 512, 512)],
                                     start=True, stop=False)
                for kt in range(KT):
                    for nch in range(2):
                        nc.tensor.matmul(ps[nch][:], lnT[:, kt, bass.ts(tt, 128)],
                                         wB[:, 2, kt, bass.ts(nch, 512)],
                                         start=False, stop=(kt == KT - 1))
                for nch in range(2):
                    dsl = v_sb[:, tt, bass.ts(nch, 512)]
                    if nch == 0:
                        nc.vector.tensor_copy(dsl, ps[nch][:])
                    else:
                        nc.scalar.activation(dsl, ps[nch][:], AF.Copy)

        def pack_kvq(b):
            nc.sync.dma_start(
                out=kvq_in[b].ap()[:, 0].rearrange("j (p t) -> p j t", p=128),
                in_=kT[:, :, bass.ds(b * TPB, TPB)])
            nc.sync.dma_start(
                out=kvq_in[b].ap()[:, 1].rearrange(
                    "j (p tt f) -> p tt j f", p=128, tt=2),
                in_=v_sb[:, 2 * b:2 * b + 2, :].rearrange(
                    "p tt (j f) -> p tt j f", j=NC))
            nc.sync.dma_start(
                out=kvq_in[b].ap()[:, 2].rearrange("j (p t) -> p j t", p=128),
                in_=qT[:, :, bass.ds(b * TPB, TPB)])
            nc.gpsimd.collective_compute(
                "AllToAll", mybir.AluOpType.bypass, replica_groups=RG,
                ins=[kvq_in[b].ap().opt()], outs=[kvq_out[b].ap().opt()])

        layer_norm_t(2 * KT, lnT, (0, 1))
        proj_qk(0)
        proj_v(0)
        pack_kvq(0)
        layer_norm_t(2 * KT, lnT, (2, 3))
        proj_qk(1)
        proj_v(1)
        pack_kvq(1)

        # =====================================================================
        # Attention: 2 heads, full sequence, exact causal, per batch
        # =====================================================================
        def attn_assemble(b):
            qa = att_in.tile([128, S], F16, tag="qa")
            ka = att_in.tile([128, S], F16, tag="ka")
            vb = att_in.tile([128, 2, NC, HPC, D + 1], F16, tag="vb")
            nc.sync.dma_start(
                out=qa[:].rearrange("p (i t) -> p i t", i=NC),
                in_=kvq_out[b].ap()[:, 2].rearrange("i (p t) -> p i t", p=128))
            nc.sync.dma_start(
                out=ka[:].rearrange("p (i t) -> p i t", i=NC),
                in_=kvq_out[b].ap()[:, 0].rearrange("i (p t) -> p i t", p=128))
            for h in range(HPC):
                for t2 in range(2):
                    nc.sync.dma_start(
                        out=vb[:, t2, :, h, 0:D],
                        in_=kvq_out[b].ap()[:, 1].rearrange(
                            "i (p tt h d) -> p i tt h d",
                            p=128, tt=2, h=HPC)[:, :, t2, h, :])
            nc.vector.memset(vb[:, :, :, :, D:D + 1], 1.0)
            return qa, ka, vb

        NQP = S // 512  # 4 query blocks per batch
        # kT / v_sb are dead once both kvq packs are sent; reuse their space
        cxu = persist.tile([D, 2 * NQP, 512], F16, tag="kT", name="cxu")
        rb_all = persist.tile([D, 2 * NQP, 512], F16, tag="v", name="rb_all")
        rows_stage = norm_pool.tile([1, 2 * NQP, 512], F32, tag="rstage")
        rsall = norm_pool.tile([2 * NQP, 512], F32, tag="rsall")
        rcp_sb = norm_pool.tile([128, 4 * 2 * NQP], F32, tag="rcp")
        rows_sb = norm_pool.tile([2 * NQP, 512], F16, tag="rows")

        def attn_compute(b, qa, ka, vb):
            for qp in range(NQP):
                nkt = 4 * (qp + 1)
                cx = [pacc.tile([D + 1, 512], F32, tag=f"acc{2 * h + qp % 2}",
                                name=f"cx{h}") for h in range(HPC)]

                def scores(kt):
                    out = []
                    for h in range(HPC):
                        sc = pmix.tile([128, 512], F32, tag="pmix", name="sc")
                        nc.tensor.matmul(
                            sc[:], ka[bass.ts(h, 64), bass.ts(kt, 128)],
                            qa[bass.ts(h, 64), bass.ts(qp, 512)],
                            start=True, stop=True)
                        out.append(sc)
                    return out

                sc_cur = scores(0)
                for kt in range(nkt):
                    sc_nxt = scores(kt + 1) if kt + 1 < nkt else None
                    atts = []
                    for h in range(HPC):
                        att = attb.tile([128, 512], F16, tag="att")
                        if kt >= 4 * qp:  # diagonal tile: relu then mask
                            nc.scalar.activation(att[:], sc_cur[h][:], AF.Relu)
                            nc.vector.tensor_tensor(
                                att[:], att[:], tri[:, kt - 4 * qp, :], MUL)
                        elif (kt + h) % 2 == 0:
                            nc.vector.tensor_scalar_max(att[:], sc_cur[h][:], 0.0)
                        else:
                            nc.scalar.activation(att[:], sc_cur[h][:], AF.Relu)
                        atts.append(att)
                    for h in range(HPC):
                        nc.tensor.matmul(cx[h][:], vb[:, kt % 2, kt // 2, h, :],
                                         atts[h][:],
                                         start=(kt == 0), stop=(kt == nkt - 1))
                    sc_cur = sc_nxt
                # evacuate unnormalized ctx + rowsum row; normalize after the
                # whole batch (one wide reciprocal instead of 8 slow row ones)
                for h in range(HPC):
                    u = 2 * qp + h
                    # engines can only write partition offsets 0/32/64/96, so
                    # stage all rows on partition 0 and restack via DMA below
                    nc.vector.tensor_copy(rows_stage[:, u, :], cx[h][D:D + 1, :])
                    nc.scalar.activation(cxu[:, u, :], cx[h][0:D, :], AF.Copy)
            # gather rowsums onto 128 partitions via PE transposes, one cheap
            # reciprocal, then broadcast back across partitions via DRAM bounce
            nc.sync.dma_start(out=rsall[:], in_=rows_stage[:])
            for c in range(4):
                rst = pmix.tile([128, 2 * NQP], F32, tag="pmix", name="rst")
                nc.tensor.transpose(rst[:], rsall[:, bass.ts(c, 128)],
                                    ident[0:2 * NQP, 0:2 * NQP])
                # eps must survive f16: all-masked row -> 0 * recip(eps) = 0
                nc.vector.tensor_scalar_add(rcp_sb[:, bass.ts(c, 2 * NQP)],
                                            rst[:], 6.5e-5)
                nc.vector.reciprocal(rcp_sb[:, bass.ts(c, 2 * NQP)],
                                     rcp_sb[:, bass.ts(c, 2 * NQP)])
            for c in range(4):
                rbk = pmix.tile([2 * NQP, 128], F32, tag="pmix", name="rbk")
                nc.tensor.transpose(rbk[:], rcp_sb[:, bass.ts(c, 2 * NQP)],
                                    ident[:])
                nc.vector.tensor_copy(rows_sb[:, bass.ts(c, 128)], rbk[:])
            nc.sync.dma_start(out=rows_dram[b].ap(), in_=rows_sb[:])
            nc.sync.dma_start(
                out=rb_all[:],
                in_=rows_dram[b].ap().unsqueeze(0).partition_broadcast(D).squeeze(1))
            for qp in range(NQP):
                for h in range(HPC):
                    u = 2 * qp + h
                    nc.vector.tensor_tensor(
                        ctxT[bass.ts(h, 64), b, bass.ts(qp, 512)],
                        cxu[:, u, :], rb_all[:, u, :], MUL)

        def pack_cc(b):
            nc.sync.dma_start(
                out=cc_in[b].ap().rearrange("j (p t) -> p j t", p=128),
                in_=ctxT[:, b, :].rearrange("p (j t) -> p j t", j=NC))
            nc.gpsimd.collective_compute(
                "AllToAll", mybir.AluOpType.bypass, replica_groups=RG,
                ins=[cc_in[b].ap().opt()], outs=[cc_out[b].ap().opt()])

        def unpack_cc(b):
            nc.sync.dma_start(
                out=ctxo[:, :, bass.ds(b * TPB, TPB)],
                in_=cc_out[b].ap().rearrange("i (p t) -> p i t", p=128))

        a0 = attn_assemble(0)
        attn_compute(0, *a0)
        a1 = attn_assemble(1)
        pack_cc(0)
        attn_compute(1, *a1)
        pack_cc(1)
        unpack_cc(0)
        unpack_cc(1)

        # =====================================================================
        # out-proj: x += ctx @ wo.T + bo
        # =====================================================================
        for half in range(2):
            for nch in range(2):
                ps = acc_tiles(n=2)
                for i, tt in enumerate((2 * half, 2 * half + 1)):
                    nc.tensor.matmul(ps[i][:], ones1[:],
                                     bias_sb[:, bass.ds(2 * H + nch * 512, 512)],
                                     start=True, stop=False)
                for kt in range(KT):
                    for i, tt in enumerate((2 * half, 2 * half + 1)):
                        nc.tensor.matmul(ps[i][:], ctxo[:, kt, bass.ts(tt, 128)],
                                         wB[:, 3, kt, bass.ts(nch, 512)],
                                         start=False, stop=(kt == KT - 1))
                for i, tt in enumerate((2 * half, 2 * half + 1)):
                    xsl = x_sb[:, tt, bass.ts(nch, 512)]
                    nc.vector.tensor_tensor(xsl, xsl, ps[i][:], ADD)

        # =====================================================================
        # FFN: x += relu(LN3(x) @ w1.T + b1f) @ w2.T + b2f
        # =====================================================================
        layer_norm_t(4 * KT, lnT, range(TT))
        h_sb = persist.tile([128, NFT, T], F16, tag="wB", name="h_sb")
        for nh in range(NFT // 4):
            ps = acc_tiles()
            for kt in range(KT):
                wt = wpool.tile([128, 512], F16, tag="wa")
                nc.sync.dma_start(
                    out=wt[:], in_=w1T_io.ap()[bass.ts(kt, 128), bass.ts(nh, 512)])
                for n4 in range(4):
                    nc.tensor.matmul(ps[n4][:], wt[:, bass.ts(n4, 128)],
                                     lnT[:, kt, :],
                                     start=(kt == 0), stop=(kt == KT - 1))
            for n4 in range(4):
                nt = nh * 4 + n4
                if n4 % 2 == 0:
                    nc.scalar.activation(h_sb[:, nt, :], ps[n4][:], AF.Relu,
                                         bias=ff1b_col[:, nt:nt + 1])
                else:
                    nc.vector.tensor_scalar(h_sb[:, nt, :], ps[n4][:],
                                            ff1b_col[:, nt:nt + 1], 0.0,
                                            op0=ADD, op1=MAX)
        for nch in range(2):
            ps = acc_tiles()
            for tt in range(TT):
                nc.tensor.matmul(ps[tt][:], ones1[:],
                                 bias_sb[:, bass.ds(3 * H + nch * 512, 512)],
                                 start=True, stop=False)
            for kt in range(NFT):
                wt = wpool.tile([128, 512], F16, tag="wa")
                nc.sync.dma_start(
                    out=wt[:], in_=w2T_io.ap()[bass.ts(kt, 128), bass.ts(nch, 512)])
                for tt in range(TT):
                    nc.tensor.matmul(ps[tt][:], h_sb[:, kt, bass.ts(tt, 128)],
                                     wt[:], start=False, stop=(kt == NFT - 1))
            for tt in range(TT):
                xsl = x_sb[:, tt, bass.ts(nch, 512)]
                nc.vector.tensor_tensor(xsl, xsl, ps[tt][:], ADD)

        # final output
        nc.sync.dma_start(out=out_io.ap().rearrange("(tt p) h -> p tt h", p=128),
                          in_=x_sb[:])

    nc.compile()
    return nc


def _prep_shared(inputs):
    f = lambda a: np.asarray(a, np.float32)
    h = lambda a: np.ascontiguousarray(a.astype(np.float16))
    sh = {
        "sfwmT": h((f(inputs["sf_w"]) * f(inputs["mask"])).T),
        "wqT": h(f(inputs["wq"]).T),
        "wkT": h(f(inputs["wk"]).T),
        "wvT": h(f(inputs["wv"]).T),
        "woT": h(f(inputs["wo"]).T),
        "w1T": h(f(inputs["ff1_w"]).T),
        "w2T": h(f(inputs["ff2_w"]).T),
    }
    sh["bias_rows"] = h(np.concatenate(
        [f(inputs["sf_b"]), f(inputs["bv"]), f(inputs["bo"]),
         f(inputs["ff2_b"])]).reshape(1, 4 * H))
    bqk = np.stack([f(inputs["bq"]), f(inputs["bk"])]) * QSC
    sh["bqk_col"] = np.ascontiguousarray(bqk.reshape(2 * KT, 128).T)
    sh["ff1b_col"] = np.ascontiguousarray(
        f(inputs["ff1_b"]).reshape(NFT, 128).T)
    gb = np.concatenate([f(inputs[k]) for k in
                         ("g1", "b1", "g2", "b2", "g3", "b3")])
    sh["gb_cols"] = np.ascontiguousarray(gb.reshape(6 * KT, 128).T)
    tri = np.zeros((4, 128, 512), np.float16)
    for d in range(4):
        for p in range(128):
            tri[d, p, 128 * d + p:] = 1.0
    sh["tri4"] = tri
    return sh


def make_in_maps(inputs):
    sh = _prep_shared(inputs)
    x = np.asarray(inputs["x"], np.float32).reshape(B, NC, TPB, H)
    in_maps = []
    for c in range(NC):
        m = dict(sh)
        m["x_c"] = np.ascontiguousarray(
            np.concatenate([x[0, c], x[1, c]], axis=0))
        in_maps.append(m)
    return in_maps


def assemble_out(results):
    out = np.empty((B, S, H), np.float32)
    for c in range(NC):
        r = results[c]["out_c"]
        out[0, c * TPB:(c + 1) * TPB] = r[:TPB]
        out[1, c * TPB:(c + 1) * TPB] = r[TPB:]
    return out


def kernel(**inputs) -> np.ndarray:
    from concourse.bass_utils import run_bass_kernel_spmd

    if "nc" not in _CACHE:
        _CACHE["nc"] = _build()
    nc = _CACHE["nc"]

    in_maps = make_in_maps(inputs)
    res = run_bass_kernel_spmd(nc, in_maps, core_ids=list(range(NC)))
    return assemble_out(res.results)
